# revision 19
# baseline (speedup 1.0000x reference)
"""Trainium2 Bass kernel for bilinear forward-warp splatting (scatter_memory).

Computes, per batch element b (data-parallel over 8 NeuronCores):
    wy = y0 + dt*fy;  wx = x0 + dt*fx          (dt = tref - i)
    out[y, x] = sum_p v_p * tent(wy_p - y) * tent(wx_p - x)
for the three channels v in {1, fy, fx}, where tent(u) = max(0, 1-|u|) is
exactly the bilinear splat weight, followed by wf = splat(w*f)/(splat(w)+eps).

Algorithm: displacement dt*flow is bounded (|d| <= DISP, verified on host), so
the scatter is a windowed rank-1 accumulation: for each chunk of 128 points
(64 rows x 2 columns of the grid), build a sparse "tent" matrix over the
y-window (lhsT [128, mY]) and x-window x 3 channels (rhs [128, XW, 3]) by
comparing warped coords against iota ramps, then let the TensorEngine
accumulate sum_p tentY[p,:]^T (x) rhs[p,:] into PSUM. PSUM segments slide
along each 64-row band and are spilled (added) into SBUF grid accumulators.
"""

import os
import sys
import math

import numpy as np

for _p in ("/opt/trn_rl_repo", "/root/.axon_site/_ro/trn_rl_repo"):
    if os.path.isdir(_p) and _p not in sys.path:
        sys.path.insert(0, _p)

from contextlib import ExitStack

import concourse.bass as bass
import concourse.bacc as bacc
import concourse.tile as tile
from concourse import mybir
from concourse.ap import AP
from concourse.bass_utils import run_bass_kernel_spmd

H, W = 480, 640
NCORES = 8
F32 = mybir.dt.float32
F16 = mybir.dt.bfloat16  # bf16: f32 exponent range (tent weights down to ~1e-9 must not flush to 0)
I32 = mybir.dt.int32
Alu = mybir.AluOpType
Act = mybir.ActivationFunctionType

BH = 64          # band height (rows per band); chunks are 64 rows x 2 cols
SP = 28          # column-pairs per PSUM segment
EPS = 1e-9
BIG = 4.0e6      # pushed onto wy for masked-out points -> tent == 0 everywhere


def _v(ap, dims, extra_off=0, parts=None):
    """Manual AP view: keep ap's partition pair, replace free dims."""
    ppair = [ap.ap[0][0], ap.ap[0][1] if parts is None else parts]
    return AP(tensor=ap.tensor, offset=ap.offset + extra_off, ap=[ppair] + [list(d) for d in dims])


def _build_program(disp, dt, H=H, W=W):
    PAD = disp + 1
    XW = 2 * disp + 4                       # x-window of one 2-col chunk
    YWMAX = BH + 2 * disp + 2               # y-window of a 64-row band
    bands = [(a, min(BH, H - a)) for a in range(0, H, BH)]
    npairs = W // 2
    nsegs = (npairs + SP - 1) // SP
    XTMAX = 2 * SP + 2 * disp + 2           # psum x extent of a full segment
    NBLK = (H + 127) // 128                 # 128-row blocks for plane storage
    # Grid accumulator rows are stored SHIFTED by +PAD ("storage row" =
    # real row + PAD) so every PSUM spill lands at a 64-aligned partition
    # start (engines require 32-aligned start partitions). Rows [0, PAD) and
    # [H+PAD, ...) of storage only ever accumulate exact zeros.
    assert PAD <= 32, "grid storage shift assumes PAD <= 32"
    NSBLK = (H + PAD + 127) // 128          # storage blocks for the grid

    nc = bacc.Bacc("TRN2", target_bir_lowering=False, debug=False)
    fy_in = nc.declare_dram_parameter("fy", [H, W], F32, isOutput=False)
    fx_in = nc.declare_dram_parameter("fx", [H, W], F32, isOutput=False)
    o_wfx = nc.declare_dram_parameter("out_wfx", [H, W], F32, isOutput=True)
    o_wfy = nc.declare_dram_parameter("out_wfy", [H, W], F32, isOutput=True)

    with ExitStack() as ctx:
        tc = ctx.enter_context(tile.TileContext(nc))
        singles = ctx.enter_context(tc.tile_pool(name="singles", bufs=1))

        # ---- constant ramps ----
        NY = H - BH + YWMAX + 8              # iotaY index range
        NX = W + XW + 8
        ioY = singles.tile([128, NY], F32)
        ioX = singles.tile([128, NX], F32)
        x0f = singles.tile([128, W], F32)
        y0f = singles.tile([128, NBLK], F32)
        ioY_i = singles.tile([128, NY], I32)
        ioX_i = singles.tile([128, NX], I32)
        x0_i = singles.tile([128, W], I32)
        y0_i = singles.tile([128, NBLK], I32)
        nc.gpsimd.iota(ioY_i[:], pattern=[[1, NY]], base=-PAD, channel_multiplier=0)
        nc.gpsimd.iota(ioX_i[:], pattern=[[1, NX]], base=-PAD, channel_multiplier=0)
        nc.gpsimd.iota(x0_i[:], pattern=[[1, W]], base=0, channel_multiplier=0)
        nc.gpsimd.iota(y0_i[:], pattern=[[128, NBLK]], base=0, channel_multiplier=1)
        nc.vector.tensor_copy(out=ioY[:], in_=ioY_i[:])
        nc.vector.tensor_copy(out=ioX[:], in_=ioX_i[:])
        nc.vector.tensor_copy(out=x0f[:], in_=x0_i[:])
        nc.vector.tensor_copy(out=y0f[:], in_=y0_i[:])

        # ---- grid accumulators (ch 0=w, 1=w*fy, 2=w*fx), +PAD row shift ----
        grid = singles.tile([128, 3, NSBLK, W], F32)
        nc.vector.memset(grid[:, 0], 0.0)
        nc.gpsimd.memset(grid[:, 1], 0.0)
        nc.vector.memset(grid[:, 2], 0.0)

        # zero operands for the per-segment PSUM-clearing matmul
        z_l = singles.tile([16, YWMAX], F16)
        z_r = singles.tile([16, XTMAX * 3], F16)
        nc.gpsimd.memset(z_l[:], 0.0)
        nc.gpsimd.memset(z_r[:], 0.0)

        # ---- load inputs & pointwise prep (parity-split planes) ----
        # PSc: plane 0 = wyM (masked warped y), plane 1 = wx   (f32)
        # PSv: plane 0 = fy, plane 1 = fx                      (f16)
        PSc = singles.tile([128, 2, NBLK, 2, W // 2], F32)
        PSv = singles.tile([128, 2, NBLK, 2, W // 2], F16)

        with tc.tile_pool(name="inpool", bufs=1) as inpool, \
             tc.tile_pool(name="preptmp", bufs=2) as preptmp:
            in_fy = inpool.tile([128, NBLK, W], F32)
            in_fx = inpool.tile([128, NBLK, W], F32)
            for blk in range(NBLK):
                rows = min(128, H - 128 * blk)
                nc.sync.dma_start(out=in_fy[:rows, blk], in_=fy_in.ap()[128 * blk:128 * blk + rows])
                nc.scalar.dma_start(out=in_fx[:rows, blk], in_=fx_in.ap()[128 * blk:128 * blk + rows])
            for blk in range(NBLK):
                rows = min(128, H - 128 * blk)
                wy = preptmp.tile([128, W], F32, tag="wy")
                wx = preptmp.tile([128, W], F32, tag="wx")
                ta = preptmp.tile([128, W], F32, tag="ta")
                tb = preptmp.tile([128, W], F32, tag="tb")
                nc.vector.tensor_scalar(out=wy[:rows], in0=in_fy[:rows, blk], scalar1=dt,
                                        scalar2=y0f[:rows, blk:blk + 1], op0=Alu.mult, op1=Alu.add)
                nc.vector.scalar_tensor_tensor(out=wx[:rows], in0=in_fx[:rows, blk], scalar=dt,
                                               in1=x0f[:rows], op0=Alu.mult, op1=Alu.add)
                # mask: inside iff min(wy, H-1-wy, wx, W-1-wx) >= 0
                nc.gpsimd.tensor_scalar(out=ta[:rows], in0=wy[:rows], scalar1=-1.0,
                                        scalar2=float(H - 1), op0=Alu.mult, op1=Alu.add)
                nc.vector.tensor_tensor(out=ta[:rows], in0=ta[:rows], in1=wy[:rows], op=Alu.min)
                nc.gpsimd.tensor_scalar(out=tb[:rows], in0=wx[:rows], scalar1=-1.0,
                                        scalar2=float(W - 1), op0=Alu.mult, op1=Alu.add)
                nc.vector.tensor_tensor(out=tb[:rows], in0=tb[:rows], in1=wx[:rows], op=Alu.min)
                nc.vector.tensor_tensor(out=ta[:rows], in0=ta[:rows], in1=tb[:rows], op=Alu.min)
                nc.gpsimd.tensor_scalar(out=ta[:rows], in0=ta[:rows], scalar1=0.0, scalar2=None,
                                        op0=Alu.is_lt)
                # parity-split writes: dest view [rows, W//2, 2] iterated (j, i), i fastest
                def split_view(t4, pl, f16=False):
                    base = t4[:rows, pl, blk]            # [rows, 2, W//2]
                    return base.rearrange("p i j -> p j i")
                nc.vector.scalar_tensor_tensor(out=split_view(PSc, 0), in0=ta[:rows], scalar=BIG,
                                               in1=wy[:rows], op0=Alu.mult, op1=Alu.add)
                nc.vector.tensor_copy(out=split_view(PSc, 1), in_=wx[:rows])
                nc.gpsimd.tensor_copy(out=split_view(PSv, 0), in_=in_fy[:rows, blk])
                nc.gpsimd.tensor_copy(out=split_view(PSv, 1), in_=in_fx[:rows, blk])

        # ---- main banded splat ----
        bandp = ctx.enter_context(tc.tile_pool(name="bandp", bufs=2))
        build = ctx.enter_context(tc.tile_pool(name="build", bufs=2))
        psump = ctx.enter_context(tc.tile_pool(name="psump", bufs=4, space="PSUM"))

        for (a, bh) in bands:
            blk, p0 = divmod(a, 128)
            mY = bh + 2 * disp + 2
            ylo = a - PAD
            bandC = bandp.tile([128, 2, W // 2], F32, tag="bandC")   # wyM, wx
            bandV = bandp.tile([128, 2, W // 2], F16, tag="bandV")   # fy, fx
            for i in (0, 1):
                nc.sync.dma_start(out=bandC[bh * i:bh * (i + 1)], in_=PSc[p0:p0 + bh, :, blk, i])
                nc.scalar.dma_start(out=bandV[bh * i:bh * (i + 1)], in_=PSv[p0:p0 + bh, :, blk, i])
            for s in range(nsegs):
                SPs = min(SP, npairs - SP * s)
                XTs = 2 * SPs + 2 * disp + 2
                xlo = 2 * SP * s - PAD
                j0 = SP * s

                uyd = build.tile([128, SP, YWMAX], F32, tag="uyd")
                tentY = build.tile([128, SP, YWMAX], F16, tag="tentY")
                uxd = build.tile([128, SP, XW], F32, tag="uxd")
                rhs = build.tile([128, SP, XW, 3], F16, tag="rhs")

                kk = 2 * bh
                # Y tents: tentY = relu(1 - |ioY - wy|)
                nc.gpsimd.tensor_tensor(
                    out=uyd[:kk, :SPs, :mY],
                    in0=_v(ioY[:, a:a + mY], [[0, SPs], [1, mY]], parts=kk),
                    in1=_v(bandC[:, 0, j0:j0 + SPs], [[1, SPs], [0, mY]], parts=kk),
                    op=Alu.subtract)
                nc.vector.scalar_tensor_tensor(out=uyd[:kk, :SPs, :mY], in0=uyd[:kk, :SPs, :mY],
                                               scalar=-1.0, in1=uyd[:kk, :SPs, :mY],
                                               op0=Alu.mult, op1=Alu.max)
                nc.scalar.activation(out=tentY[:kk, :SPs, :mY], in_=uyd[:kk, :SPs, :mY],
                                     func=Act.Relu, scale=-1.0, bias=1.0)
                # X tents: rhs0 = relu(1 - |ioX - wx|) (computed in f32, rounded
                # only at the final small tent value)
                nc.gpsimd.tensor_tensor(
                    out=uxd[:kk, :SPs, :],
                    in0=_v(ioX[:, 2 * j0:2 * j0 + XW], [[2, SPs], [1, XW]], parts=kk),
                    in1=_v(bandC[:, 1, j0:j0 + SPs], [[1, SPs], [0, XW]], parts=kk),
                    op=Alu.subtract)
                nc.vector.scalar_tensor_tensor(out=uxd[:kk, :SPs, :], in0=uxd[:kk, :SPs, :],
                                               scalar=-1.0, in1=uxd[:kk, :SPs, :],
                                               op0=Alu.mult, op1=Alu.max)
                nc.scalar.activation(out=rhs[:kk, :SPs, :, 0], in_=uxd[:kk, :SPs, :],
                                     func=Act.Relu, scale=-1.0, bias=1.0)
                nc.vector.tensor_tensor(out=rhs[:kk, :SPs, :, 1], in0=rhs[:kk, :SPs, :, 0],
                                        in1=_v(bandV[:, 0, j0:j0 + SPs], [[1, SPs], [0, XW]], parts=kk),
                                        op=Alu.mult)
                nc.vector.tensor_tensor(out=rhs[:kk, :SPs, :, 2], in0=rhs[:kk, :SPs, :, 0],
                                        in1=_v(bandV[:, 1, j0:j0 + SPs], [[1, SPs], [0, XW]], parts=kk),
                                        op=Alu.mult)

                pseg = psump.tile([128, XTMAX * 3], F32, tag="pseg")
                # start=True zero matmul: clears the bank's has_written bits and
                # writes 0 over the full extent, so the sliding accumulation
                # below is well-defined per element.
                nc.tensor.matmul(pseg[:mY, :XTs * 3], lhsT=z_l[:, :mY],
                                 rhs=z_r[:, :XTs * 3], start=True, stop=False)
                for jj in range(SPs):
                    nc.tensor.matmul(
                        pseg[:mY, 6 * jj:6 * jj + XW * 3],
                        lhsT=tentY[:kk, jj, :mY],
                        rhs=rhs[:kk, jj].rearrange("p a b -> p (a b)"),
                        start=False, stop=(jj == SPs - 1))

                # spill: add psum into the shifted grid (storage row = real+PAD;
                # window storage span is [a, a+mY) -> 64-aligned pieces)
                c0 = max(0, xlo)
                c1 = min(W, xlo + XTs)
                ncols = c1 - c0
                s0 = a
                s1 = min(a + mY, H + PAD)
                y = s0
                while y < s1:
                    gblk, gp = divmod(y, 128)
                    ln = min(s1 - y, 128 - gp)
                    pr = y - a
                    nc.vector.tensor_tensor(
                        out=grid[gp:gp + ln, :, gblk, c0:c1],
                        in0=_v(pseg[pr:pr + ln, :], [[1, 3], [3, ncols]],
                               extra_off=(c0 - xlo) * 3),
                        in1=grid[gp:gp + ln, :, gblk, c0:c1],
                        op=Alu.add)
                    y += ln

        # ---- normalize + store (undo the +PAD storage shift in the DMA) ----
        rec = singles.tile([128, NSBLK, W], F32)
        ofx = singles.tile([128, NSBLK, W], F32)
        ofy = singles.tile([128, NSBLK, W], F32)
        nc.vector.tensor_scalar(out=rec[:], in0=grid[:, 0], scalar1=EPS, scalar2=None, op0=Alu.add)
        nc.vector.reciprocal(out=rec[:], in_=rec[:])
        nc.vector.tensor_tensor(out=ofx[:], in0=grid[:, 2], in1=rec[:], op=Alu.mult)
        nc.vector.tensor_tensor(out=ofy[:], in0=grid[:, 1], in1=rec[:], op=Alu.mult)
        y = PAD  # storage row of real row 0
        while y < H + PAD:
            gblk, gp = divmod(y, 128)
            ln = min(H + PAD - y, 128 - gp)
            r0 = y - PAD
            nc.sync.dma_start(out=o_wfx.ap()[r0:r0 + ln], in_=ofx[gp:gp + ln, gblk])
            nc.scalar.dma_start(out=o_wfy.ap()[r0:r0 + ln], in_=ofy[gp:gp + ln, gblk])
            y += ln

    nc.compile()
    return nc


_PROG_CACHE = {}


def _get_program(disp, dt):
    key = (disp, float(dt))
    if key not in _PROG_CACHE:
        _PROG_CACHE[key] = _build_program(disp, dt)
    return _PROG_CACHE[key]


def kernel(flow_maps_x, flow_maps_y, i=0, tref=4):
    i = int(i)
    tref = int(tref)
    dt = float(tref - i)
    B = flow_maps_x.shape[0]
    assert B <= NCORES, f"batch {B} > {NCORES} cores not supported"
    fx = np.ascontiguousarray(flow_maps_x[:, i]).astype(np.float32)
    fy = np.ascontiguousarray(flow_maps_y[:, i]).astype(np.float32)

    dmax = abs(dt) * max(float(np.abs(fx).max()), float(np.abs(fy).max()))
    disp = max(2, int(math.ceil(dmax)) + 1)
    disp = max(disp, 23)  # canonical window; recompiles only if data exceeds it

    nc = _get_program(disp, dt)
    in_maps = [{"fy": fy[b], "fx": fx[b]} for b in range(B)]
    res = run_bass_kernel_spmd(nc, in_maps, list(range(B)))
    wfx = np.stack([res.results[b]["out_wfx"] for b in range(B)])[:, None]
    wfy = np.stack([res.results[b]["out_wfy"] for b in range(B)])[:, None]
    return wfx.astype(np.float32), wfy.astype(np.float32)


def _ensure_ntff_hook():
    """The agent image lacks antenv.axon_hooks; synthesize it from trn_agent_boot."""
    import types
    try:
        import antenv.axon_hooks  # noqa: F401
        return
    except ImportError:
        pass
    from trn_agent_boot.trn_boot import _ntff_profile_via_ctypes
    hook = _ntff_profile_via_ctypes("/opt/axon/libaxon_pjrt.so")
    m = types.ModuleType("antenv.axon_hooks")
    m.get_axon_ntff_profile_hook = lambda: hook
    m.set_axon_ntff_profile_hook = lambda h: None
    sys.modules["antenv.axon_hooks"] = m


def timed_run(np_inputs):
    """Run once with NTFF tracing; return HW exec time in ns (max over traced cores)."""
    _ensure_ntff_hook()
    i = int(np_inputs["i"]); tref = int(np_inputs["tref"])
    dt = float(tref - i)
    fx = np.ascontiguousarray(np_inputs["flow_maps_x"][:, i]).astype(np.float32)
    fy = np.ascontiguousarray(np_inputs["flow_maps_y"][:, i]).astype(np.float32)
    B = fx.shape[0]
    nc = _get_program(23, dt)
    in_maps = [{"fy": fy[b], "fx": fx[b]} for b in range(B)]
    res = run_bass_kernel_spmd(nc, in_maps, list(range(B)), trace=True)
    return res.exec_time_ns


if __name__ == "__main__":
    rng = np.random.default_rng(0)
    fmx = rng.standard_normal((8, 4, H, W), dtype=np.float32)
    fmy = rng.standard_normal((8, 4, H, W), dtype=np.float32)
    ox, oy = kernel(fmx, fmy, 0, 4)
    print(ox.shape, oy.shape, ox.dtype)


# revision 20
# speedup vs baseline: 1.9911x; 1.9911x over previous
"""Trainium2 Bass kernel for bilinear forward-warp splatting (scatter_memory).

Computes, per batch element b (data-parallel over 8 NeuronCores):
    wy = y0 + dt*fy;  wx = x0 + dt*fx          (dt = tref - i)
    out[y, x] = sum_p v_p * tent(wy_p - y) * tent(wx_p - x)
for the three channels v in {1, fy, fx}, where tent(u) = max(0, 1-|u|) is
exactly the bilinear splat weight, followed by wf = splat(w*f)/(splat(w)+eps).

Algorithm: displacement dt*flow is bounded (verified on host per region), so
the scatter is a windowed rank-1 accumulation: for each chunk of 128 points
(64 rows x 2 columns of the grid), build a sparse "tent" matrix over the
y-window (lhsT [128, mY]) and the x-window x 3 channels (rhs) with a single
fused custom-DVE op (relu(1-|iota-w|)), then let the TensorEngine accumulate
sum_p tentY[p,:]^T (x) rhs[p,:] into PSUM, sliding along 64-row bands.
PSUM segments are spilled (added) into SBUF grid accumulators.
"""

import os
import sys
import math

import numpy as np

for _p in ("/opt/trn_rl_repo", "/root/.axon_site/_ro/trn_rl_repo"):
    if os.path.isdir(_p) and _p not in sys.path:
        sys.path.insert(0, _p)

from contextlib import ExitStack

import concourse.bass as bass
import concourse.bacc as bacc
import concourse.tile as tile
from concourse import mybir
from concourse.ap import AP
from concourse.bass_utils import run_bass_kernel_spmd

H, W = 480, 640
NCORES = 8
F32 = mybir.dt.float32
BF16 = mybir.dt.bfloat16  # bf16: f32 exponent range (tent weights down to ~1e-9 must not flush to 0)
Alu = mybir.AluOpType
Act = mybir.ActivationFunctionType

BH = 64          # band height (rows per band); chunks are 64 rows x 2 cols
SP = 56          # column-pairs per PSUM segment
EPS = 1e-9
BIG = 4.0e6      # pushed onto wy for masked-out points -> tent == 0 everywhere

_TENT_OP = None


def _tent_op():
    """Register (once) the fused tent op: out = relu(1 - |in0 - in1|)."""
    global _TENT_OP
    if _TENT_OP is not None:
        return _TENT_OP
    from concourse import dve_ops as dvo
    from concourse.dve_spec import Spec, Src0, Src1, One, maxx, relu, lower
    from concourse.dve_uop import DveOpSpec

    name = "TENT_ANT"
    for op in dvo.OPS:
        if op.name == name:
            _TENT_OP = op
            return op
    spec = Spec(
        body=relu(One - maxx(Src0 - Src1, Src1 - Src0)),
        reference=lambda in0, in1, s0, s1, imm2: np.maximum(
            0.0, 1.0 - np.abs(in0 - in1)
        ),
    )
    row = dvo._CUSTOM_DVE_ROW_BASE + len(dvo.OPS)
    shas = {}
    for ver in ("v3", "v4"):
        shas[ver] = DveOpSpec(
            name=name, opcode=row, uops=lower(spec, ver=ver), rd1_en=True
        ).sha(ver)
    op = dvo.DveOp(name, spec, subdim=False, uops_sha=shas)
    dvo.OPS.append(op)
    dvo._SUB_OPCODE_FOR_NAME[name] = row
    dvo.CUSTOM_DVE_SPECS[name] = spec
    _TENT_OP = op
    return op


def _v(ap, dims, extra_off=0, parts=None):
    """Manual AP view: keep ap's partition pair, replace free dims."""
    ppair = [ap.ap[0][0], ap.ap[0][1] if parts is None else parts]
    return AP(tensor=ap.tensor, offset=ap.offset + extra_off, ap=[ppair] + [list(d) for d in dims])


def _build_program(disp, dt, dx_map, H=H, W=W):
    """disp: global y half-window; dx_map[band][seg]: x half-window per region."""
    TENT = _tent_op()
    PAD = disp + 1
    YWMAX = BH + 2 * disp + 2               # y-window of a 64-row band
    bands = [(a, min(BH, H - a)) for a in range(0, H, BH)]
    npairs = W // 2
    nsegs = (npairs + SP - 1) // SP
    dxmax = max(max(r) for r in dx_map)
    XWMAX = 2 * dxmax + 4
    XT3MAX = (2 * SP + 2 * dxmax + 2) * 3   # psum extent (ch-inner) of a segment
    assert XT3MAX <= 512
    NBLK = (H + 127) // 128                 # 128-row blocks for plane storage
    # Grid accumulator rows are stored SHIFTED by +PAD ("storage row" =
    # real row + PAD) so every PSUM spill lands at a 64-aligned partition
    # start (engines require 32-aligned start partitions). Rows [0, PAD) and
    # [H+PAD, ...) of storage only ever accumulate exact zeros.
    assert PAD <= 32, "grid storage shift assumes PAD <= 32"
    NSBLK = (H + PAD + 127) // 128          # storage blocks for the grid
    assert len(dx_map) == len(bands) and all(len(r) == nsegs for r in dx_map)

    nc = bacc.Bacc("TRN2", target_bir_lowering=False, debug=False)
    fy_in = nc.declare_dram_parameter("fy", [H, W], F32, isOutput=False)
    fx_in = nc.declare_dram_parameter("fx", [H, W], F32, isOutput=False)
    o_wfx = nc.declare_dram_parameter("out_wfx", [H, W], F32, isOutput=True)
    o_wfy = nc.declare_dram_parameter("out_wfy", [H, W], F32, isOutput=True)

    with ExitStack() as ctx:
        tc = ctx.enter_context(tile.TileContext(nc))
        singles = ctx.enter_context(tc.tile_pool(name="singles", bufs=1))

        # ---- constant ramps (f32 iotas: all values exact below 2^24) ----
        NY = H - BH + YWMAX + 8
        NX = W + XWMAX + 8
        ioY = singles.tile([128, NY], F32)
        ioX = singles.tile([128, NX], F32)
        x0f = singles.tile([128, W], F32)
        y0f = singles.tile([128, NBLK], F32)
        nc.gpsimd.iota(ioY[:], pattern=[[1, NY]], base=-PAD, channel_multiplier=0,
                       allow_small_or_imprecise_dtypes=True)
        nc.gpsimd.iota(ioX[:], pattern=[[1, NX]], base=-(dxmax + 1), channel_multiplier=0,
                       allow_small_or_imprecise_dtypes=True)
        nc.gpsimd.iota(x0f[:], pattern=[[1, W]], base=0, channel_multiplier=0,
                       allow_small_or_imprecise_dtypes=True)
        nc.gpsimd.iota(y0f[:], pattern=[[128, NBLK]], base=0, channel_multiplier=1,
                       allow_small_or_imprecise_dtypes=True)

        # ---- grid accumulators (ch 0=w, 1=w*fy, 2=w*fx), +PAD row shift ----
        grid = singles.tile([128, 3, NSBLK, W], F32)
        nc.vector.memset(grid[:, 0], 0.0)
        nc.gpsimd.memset(grid[:, 1], 0.0)
        nc.vector.memset(grid[:, 2], 0.0)

        # zero operands for the per-segment PSUM-clearing matmul
        z_l = singles.tile([16, YWMAX], BF16)
        z_r = singles.tile([16, 512], BF16)
        nc.gpsimd.memset(z_l[:], 0.0)
        nc.gpsimd.memset(z_r[:], 0.0)

        # ---- load inputs & pointwise prep (parity-split planes) ----
        # PSc: plane 0 = wyM (masked warped y), plane 1 = wx   (f32)
        # PSv: plane 0 = fy, plane 1 = fx                      (bf16)
        PSc = singles.tile([128, 2, NBLK, 2, W // 2], F32)
        PSv = singles.tile([128, 2, NBLK, 2, W // 2], BF16)

        with tc.tile_pool(name="inpool", bufs=1) as inpool, \
             tc.tile_pool(name="preptmp", bufs=2) as preptmp:
            in_fy = inpool.tile([128, NBLK, W], F32)
            in_fx = inpool.tile([128, NBLK, W], F32)
            for blk in range(NBLK):
                rows = min(128, H - 128 * blk)
                nc.sync.dma_start(out=in_fy[:rows, blk], in_=fy_in.ap()[128 * blk:128 * blk + rows])
                nc.scalar.dma_start(out=in_fx[:rows, blk], in_=fx_in.ap()[128 * blk:128 * blk + rows])
            for blk in range(NBLK):
                rows = min(128, H - 128 * blk)
                wy = preptmp.tile([128, W], F32, tag="wy")
                wx = preptmp.tile([128, W], F32, tag="wx")
                ta = preptmp.tile([128, W], F32, tag="ta")
                tb = preptmp.tile([128, W], F32, tag="tb")
                nc.vector.tensor_scalar(out=wy[:rows], in0=in_fy[:rows, blk], scalar1=dt,
                                        scalar2=y0f[:rows, blk:blk + 1], op0=Alu.mult, op1=Alu.add)
                nc.vector.scalar_tensor_tensor(out=wx[:rows], in0=in_fx[:rows, blk], scalar=dt,
                                               in1=x0f[:rows], op0=Alu.mult, op1=Alu.add)
                # mask: inside iff min(wy, H-1-wy, wx, W-1-wx) >= 0
                nc.gpsimd.tensor_scalar(out=ta[:rows], in0=wy[:rows], scalar1=-1.0,
                                        scalar2=float(H - 1), op0=Alu.mult, op1=Alu.add)
                nc.vector.tensor_tensor(out=ta[:rows], in0=ta[:rows], in1=wy[:rows], op=Alu.min)
                nc.gpsimd.tensor_scalar(out=tb[:rows], in0=wx[:rows], scalar1=-1.0,
                                        scalar2=float(W - 1), op0=Alu.mult, op1=Alu.add)
                nc.vector.tensor_tensor(out=tb[:rows], in0=tb[:rows], in1=wx[:rows], op=Alu.min)
                nc.vector.tensor_tensor(out=ta[:rows], in0=ta[:rows], in1=tb[:rows], op=Alu.min)
                nc.vector.tensor_scalar(out=ta[:rows], in0=ta[:rows], scalar1=0.0, scalar2=None,
                                        op0=Alu.is_lt)
                # parity-split writes: dest view [rows, W//2, 2] iterated (j, i), i fastest
                def split_view(t4, pl):
                    return t4[:rows, pl, blk].rearrange("p i j -> p j i")
                nc.vector.scalar_tensor_tensor(out=split_view(PSc, 0), in0=ta[:rows], scalar=BIG,
                                               in1=wy[:rows], op0=Alu.mult, op1=Alu.add)
                nc.vector.tensor_copy(out=split_view(PSc, 1), in_=wx[:rows])
                nc.gpsimd.tensor_copy(out=split_view(PSv, 0), in_=in_fy[:rows, blk])
                nc.gpsimd.tensor_copy(out=split_view(PSv, 1), in_=in_fx[:rows, blk])

        # ---- main banded splat ----
        bandp = ctx.enter_context(tc.tile_pool(name="bandp", bufs=2))
        build = ctx.enter_context(tc.tile_pool(name="build", bufs=2))
        psump = ctx.enter_context(tc.tile_pool(name="psump", bufs=4, space="PSUM"))

        for bi, (a, bh) in enumerate(bands):
            blk, p0 = divmod(a, 128)
            mY = bh + 2 * disp + 2
            kk = 2 * bh
            bandC = bandp.tile([128, 2, W // 2], F32, tag="bandC")   # wyM, wx
            bandV = bandp.tile([128, 2, W // 2], BF16, tag="bandV")  # fy, fx
            for i in (0, 1):
                nc.sync.dma_start(out=bandC[bh * i:bh * (i + 1)], in_=PSc[p0:p0 + bh, :, blk, i])
                nc.scalar.dma_start(out=bandV[bh * i:bh * (i + 1)], in_=PSv[p0:p0 + bh, :, blk, i])
            for s in range(nsegs):
                SPs = min(SP, npairs - SP * s)
                dx = dx_map[bi][s]
                XW = 2 * dx + 4
                XTs = 2 * SPs + 2 * dx + 2
                xlo = 2 * SP * s - (dx + 1)
                j0 = SP * s

                tentY = build.tile([128, SP, YWMAX], BF16, tag="tentY")
                rhs = build.tile([128, SP, 3, XWMAX], BF16, tag="rhs")

                # Y tents: tentY = relu(1 - |ioY - wy|), one fused DVE pass
                nc.vector._custom_dve(
                    TENT,
                    out=tentY[:kk, :SPs, :mY],
                    in0=_v(ioY[:, a:a + mY], [[0, SPs], [1, mY]], parts=kk),
                    in1=_v(bandC[:, 0, j0:j0 + SPs], [[1, SPs], [0, mY]], parts=kk))
                # X tents into rhs channel 0 (contiguous)
                nc.vector._custom_dve(
                    TENT,
                    out=rhs[:kk, :SPs, 0, :XW],
                    in0=_v(ioX[:, 2 * j0 + dxmax - dx:], [[2, SPs], [1, XW]], parts=kk),
                    in1=_v(bandC[:, 1, j0:j0 + SPs], [[1, SPs], [0, XW]], parts=kk))
                nc.vector.tensor_tensor(out=rhs[:kk, :SPs, 1, :XW], in0=rhs[:kk, :SPs, 0, :XW],
                                        in1=_v(bandV[:, 0, j0:j0 + SPs], [[1, SPs], [0, XW]], parts=kk),
                                        op=Alu.mult)
                nc.vector.tensor_tensor(out=rhs[:kk, :SPs, 2, :XW], in0=rhs[:kk, :SPs, 0, :XW],
                                        in1=_v(bandV[:, 1, j0:j0 + SPs], [[1, SPs], [0, XW]], parts=kk),
                                        op=Alu.mult)

                pseg = psump.tile([128, XT3MAX], F32, tag="pseg")
                # start=True zero matmul: clears the bank's has_written bits and
                # writes 0 over the full extent, so the sliding accumulation
                # below is well-defined per element.
                nc.tensor.matmul(pseg[:mY, :XTs * 3], lhsT=z_l[:, :mY],
                                 rhs=z_r[:, :XTs * 3], start=True, stop=False)
                for jj in range(SPs):
                    # rhs chunk read ch-inner (x outer, ch inner) to match psum
                    rhs_j = _v(rhs[:kk], [[1, XW], [XWMAX, 3]],
                               extra_off=jj * 3 * XWMAX)
                    nc.tensor.matmul(
                        pseg[:mY, 6 * jj:6 * jj + XW * 3],
                        lhsT=tentY[:kk, jj, :mY],
                        rhs=rhs_j,
                        start=False, stop=(jj == SPs - 1))

                # spill: add psum into the shifted grid (storage row = real+PAD;
                # window storage span is [a, a+mY) -> 64-aligned pieces)
                c0 = max(0, xlo)
                c1 = min(W, xlo + XTs)
                ncols = c1 - c0
                s1 = min(a + mY, H + PAD)
                y = a
                while y < s1:
                    gblk, gp = divmod(y, 128)
                    ln = min(s1 - y, 128 - gp)
                    pr = y - a
                    nc.vector.tensor_tensor(
                        out=grid[gp:gp + ln, :, gblk, c0:c1],
                        in0=_v(pseg[pr:pr + ln, :], [[1, 3], [3, ncols]],
                               extra_off=(c0 - xlo) * 3),
                        in1=grid[gp:gp + ln, :, gblk, c0:c1],
                        op=Alu.add)
                    y += ln

        # ---- normalize + store (undo the +PAD storage shift in the DMA) ----
        rec = singles.tile([128, NSBLK, W], F32)
        ofx = singles.tile([128, NSBLK, W], F32)
        ofy = singles.tile([128, NSBLK, W], F32)
        nc.vector.tensor_scalar(out=rec[:], in0=grid[:, 0], scalar1=EPS, scalar2=None, op0=Alu.add)
        nc.vector.reciprocal(out=rec[:], in_=rec[:])
        nc.vector.tensor_tensor(out=ofx[:], in0=grid[:, 2], in1=rec[:], op=Alu.mult)
        nc.vector.tensor_tensor(out=ofy[:], in0=grid[:, 1], in1=rec[:], op=Alu.mult)
        y = PAD  # storage row of real row 0
        while y < H + PAD:
            gblk, gp = divmod(y, 128)
            ln = min(H + PAD - y, 128 - gp)
            r0 = y - PAD
            nc.sync.dma_start(out=o_wfx.ap()[r0:r0 + ln], in_=ofx[gp:gp + ln, gblk])
            nc.scalar.dma_start(out=o_wfy.ap()[r0:r0 + ln], in_=ofy[gp:gp + ln, gblk])
            y += ln

    nc.compile()
    return nc


_PROG_CACHE = {}


def _get_program(disp, dt, dx_map, H=H, W=W):
    key = (disp, float(dt), tuple(tuple(r) for r in dx_map), H, W)
    if key not in _PROG_CACHE:
        _PROG_CACHE[key] = _build_program(disp, dt, dx_map, H=H, W=W)
    return _PROG_CACHE[key]


def _window_params(fy, fx, dt, H=H, W=W):
    """Exact per-region displacement bounds (over all batch elements)."""
    ady = np.abs(dt) * np.abs(fy).max(axis=0)      # [H, W]
    adx = np.abs(dt) * np.abs(fx).max(axis=0)
    disp = max(2, int(math.ceil(float(ady.max()))))
    bands = [(a, min(BH, H - a)) for a in range(0, H, BH)]
    npairs = W // 2
    nsegs = (npairs + SP - 1) // SP
    dx_map = []
    for (a, bh) in bands:
        row = []
        for s in range(nsegs):
            c0 = 2 * SP * s
            c1 = min(W, 2 * SP * (s + 1))
            m = float(adx[a:a + bh, c0:c1].max())
            row.append(max(2, int(math.ceil(m))))
        dx_map.append(row)
    return disp, dx_map


def kernel(flow_maps_x, flow_maps_y, i=0, tref=4):
    i = int(i)
    tref = int(tref)
    dt = float(tref - i)
    B = flow_maps_x.shape[0]
    assert B <= NCORES, f"batch {B} > {NCORES} cores not supported"
    fx = np.ascontiguousarray(flow_maps_x[:, i]).astype(np.float32)
    fy = np.ascontiguousarray(flow_maps_y[:, i]).astype(np.float32)

    disp, dx_map = _window_params(fy, fx, dt)
    nc = _get_program(disp, dt, dx_map)
    in_maps = [{"fy": fy[b], "fx": fx[b]} for b in range(B)]
    res = run_bass_kernel_spmd(nc, in_maps, list(range(B)))
    wfx = np.stack([res.results[b]["out_wfx"] for b in range(B)])[:, None]
    wfy = np.stack([res.results[b]["out_wfy"] for b in range(B)])[:, None]
    return wfx.astype(np.float32), wfy.astype(np.float32)


def _ensure_ntff_hook():
    """The agent image lacks antenv.axon_hooks; synthesize it from trn_agent_boot."""
    import types
    try:
        import antenv.axon_hooks  # noqa: F401
        return
    except ImportError:
        pass
    from trn_agent_boot.trn_boot import _ntff_profile_via_ctypes
    hook = _ntff_profile_via_ctypes("/opt/axon/libaxon_pjrt.so")
    m = types.ModuleType("antenv.axon_hooks")
    m.get_axon_ntff_profile_hook = lambda: hook
    m.set_axon_ntff_profile_hook = lambda h: None
    sys.modules["antenv.axon_hooks"] = m


def timed_run(np_inputs):
    """Run once with NTFF tracing; return HW exec time in ns (max over traced cores)."""
    _ensure_ntff_hook()
    i = int(np_inputs["i"]); tref = int(np_inputs["tref"])
    dt = float(tref - i)
    fx = np.ascontiguousarray(np_inputs["flow_maps_x"][:, i]).astype(np.float32)
    fy = np.ascontiguousarray(np_inputs["flow_maps_y"][:, i]).astype(np.float32)
    B = fx.shape[0]
    disp, dx_map = _window_params(fy, fx, dt)
    nc = _get_program(disp, dt, dx_map)
    in_maps = [{"fy": fy[b], "fx": fx[b]} for b in range(B)]
    res = run_bass_kernel_spmd(nc, in_maps, list(range(B)), trace=True)
    return res.exec_time_ns


if __name__ == "__main__":
    rng = np.random.default_rng(0)
    fmx = rng.standard_normal((8, 4, H, W), dtype=np.float32)
    fmy = rng.standard_normal((8, 4, H, W), dtype=np.float32)
    ox, oy = kernel(fmx, fmy, 0, 4)
    print(ox.shape, oy.shape, ox.dtype)
